# revision 21
# baseline (speedup 1.0000x reference)
"""Hodgkin-Huxley cable integrator for Trainium2 (Bass/Tile).

kernel(**inputs) takes the FULL inputs of reference.setup_inputs() and returns
the tuple reference() returns:
    (mem_currents [T,N], mem_voltages [T,N], Vf [N], mf [N], hf [N], nf [N])

Design: time is strictly sequential (8192 Euler steps) over only N=2048
segments ([128 partitions x 16 free]), so per-step instruction overhead — not
bandwidth — dominates.  Cross-core halo exchange per step (~10us floor) dwarfs
the ~2us step budget, so all 8 cores run the identical SPMD program and core
0's output is used.  Within one core:

  * per-step voltage state lives in width-18 blocks (halo|16 data|halo) so the
    cable stencil is two shifted free-dim reads;
  * halos refresh each step via 4 tiny shift matmuls on the (otherwise idle)
    tensor engine implementing exact sealed-end edge-pad boundaries;
  * 5 exponentials + 1 tanh (sigmoid via tanh — same ACT table set as exp, so
    no table switches) run on the scalar engine straight from V, with all
    output scale factors (4, 0.07, 0.125, DT) folded into the exp bias;
  * remaining elementwise work is ~20 DVE ops per step using fused
    tensor_scalar affines plus custom fused DVE uop programs;
  * uniform conductances (the graded inputs are uniform) fold into immediates.

Non-uniform conductance inputs fall back to a numpy reference implementation.
"""

import math

import numpy as np

T_FULL, N_SEG = 8192, 2048
P, F = 128, 16
BLK = F + 2
DT = 0.025
E_NA, E_K, E_L = 50.0, -77.0, -54.387

_PROGRAM_CACHE = {}


def _register_custom_ops():
    from concourse.dve_spec import Spec, Src0, Src1, sq, One, C0, lower, _has_src1
    from concourse import dve_ops as DO
    from concourse.dve_uop import DveOpSpec

    defs = {
        "HH_AXS": Spec(  # g_ax * (V[s-1] + V[s+1])
            body=(Src0 + Src1) * C0,
            reference=lambda in0, in1, s0, s1, imm2: (in0 + in1) * s0,
        ),
        "HH_GATE_DECAY": Spec(  # gate * (1 - S)
            body=Src0 * (One - Src1),
            reference=lambda in0, in1, s0, s1, imm2: in0 * (1.0 - in1),
        ),
        "HH_M3HG": Spec(  # gna * m^3 * h
            body=(sq(Src0) * (Src0 * Src1)) * C0,
            reference=lambda in0, in1, s0, s1, imm2: (in0 * in0) * (in0 * in1) * s0,
        ),
        "HH_N4G": Spec(  # gk * n^4
            body=sq(sq(Src0)) * C0,
            reference=lambda in0, in1, s0, s1, imm2: (in0 * in0) ** 2 * s0,
        ),
    }
    ops = {}
    for name, spec in defs.items():
        if name in DO._SUB_OPCODE_FOR_NAME:
            ops[name] = next(o for o in DO.OPS if o.name == name)
            continue
        row = max(DO._SUB_OPCODE_FOR_NAME.values()) + 1
        assert row < 0x20
        shas = {}
        for ver in ("v3", "v4"):
            tmp = DveOpSpec(name=name, opcode=row, uops=lower(spec, ver=ver),
                            rd1_en=_has_src1(spec))
            shas[ver] = tmp.sha(ver)
        op = DO.DveOp(name, spec, subdim=False, uops_sha=shas)
        DO.OPS.append(op)
        DO._SUB_OPCODE_FOR_NAME[name] = row
        DO.CUSTOM_DVE_SPECS[name] = spec
        ops[name] = op
    return ops


def _split_drain_waits(nc):
    """This walrus rejects instructions carrying more than one sync wait;
    split excess waits onto inserted same-engine Drain carriers."""
    from concourse import mybir

    k = [0]
    for f in nc.m.functions:
        for b in f.blocks:
            out = []
            for ins in b.instructions:
                si = ins.sync_info
                if si is not None and si.on_wait and len(si.on_wait) > 1:
                    waits = list(si.on_wait)
                    for w in waits[:-1]:
                        k[0] += 1
                        out.append(mybir.InstDrain(
                            name=f"{ins.name}-wsplit{k[0]}", engine=ins.engine,
                            ins=[], outs=[],
                            sync_info=mybir.SyncInfo(on_wait=[w], on_update=[]),
                        ))
                    ins.sync_info = mybir.SyncInfo(
                        on_wait=[waits[-1]], on_update=list(si.on_update))
                out.append(ins)
            b.instructions = out


def _register_bias_const(nc, value):
    from concourse import mybir

    key = (mybir.dt.float32, float(value))
    if key in nc.const_aps.aps:
        return
    t = nc.alloc_sbuf_tensor(f"const-float32-{value}", [128, 1], mybir.dt.float32)
    nc.gpsimd.memset(t.ap(), float(value))
    nc.const_aps.aps[key] = t.ap()
    nc.all_engine_barrier()


def _build_program(scal, T_total, UT, repeat=1, loop=True):
    """scal = dict(gna, gk, gl, gax) floats.  T_total % UT == 0, UT even.
    repeat>1 re-runs the whole integration (benchmark use only).
    loop=False unrolls chunk iterations statically (simulation use)."""
    from contextlib import ExitStack

    import concourse.bass as bass
    import concourse.tile as tile
    from concourse import mybir

    AluOp = mybir.AluOpType
    AF = mybir.ActivationFunctionType
    f32 = mybir.dt.float32
    n_iter = T_total // UT
    assert n_iter * UT == T_total and UT % 2 == 0

    gna, gk, gl, gax = (float(scal[k]) for k in ("gna", "gk", "gl", "gax"))
    ln = math.log
    EXPS = [
        ("e1", -0.1, -4.0),
        ("e5", -0.1, -5.5),
        ("DTah", -1.0 / 20, ln(0.07 * DT) - 65.0 / 20),
        ("DTbm", -1.0 / 18, ln(4.0 * DT) - 65.0 / 18),
        ("DTbn", -1.0 / 80, ln(0.125 * DT) - 65.0 / 80),
    ]

    nc = bass.Bass("TRN2")
    # activation float biases lower to per-partition const APs, which must be
    # pre-registered (Bass only registers 0.0/1.0)
    for _, _sc, _bias in EXPS:
        _register_bias_const(nc, _bias)
    _register_bias_const(nc, 1.75)

    stim = nc.dram_tensor("stim", (P, T_total, F), f32, kind="ExternalInput")
    v0 = nc.dram_tensor("v0", (P, F), f32, kind="ExternalInput")
    m0 = nc.dram_tensor("m0", (P, 48), f32, kind="ExternalInput")  # [m|n|h]
    shifts = nc.dram_tensor("shifts", (P, 4 * P), f32, kind="ExternalInput")
    mc_out = nc.dram_tensor("mc", (P, T_total, F), f32, kind="ExternalOutput")
    v_out = nc.dram_tensor("vout", (P, T_total, F), f32, kind="ExternalOutput")
    vf_out = nc.dram_tensor("vf", (P, F), f32, kind="ExternalOutput")
    mf_out = nc.dram_tensor("mf", (P, 48), f32, kind="ExternalOutput")

    with ExitStack() as ctx:
        tc = ctx.enter_context(tile.TileContext(nc))
        const = ctx.enter_context(tc.tile_pool(name="const", bufs=1))
        state = ctx.enter_context(tc.tile_pool(name="state", bufs=1))
        iop = ctx.enter_context(tc.tile_pool(name="io", bufs=1))
        wsp = ctx.enter_context(tc.tile_pool(name="ws", bufs=4))
        psp = ctx.enter_context(tc.tile_pool(name="ps", bufs=4, space="PSUM"))

        sh = const.tile([P, 4 * P], f32)
        nc.gpsimd.dma_start(out=sh, in_=shifts[:, :])
        SH_L1, SH_E00 = sh[:, 0:P], sh[:, P:2 * P]
        SH_R1, SH_E127 = sh[:, 2 * P:3 * P], sh[:, 3 * P:4 * P]

        Vst = state.tile([P, (UT + 1) * BLK], f32)
        M = state.tile([P, 96], f32)            # ping-pong [m|n|h] x2
        stim_t = iop.tile([P, UT * F], f32)
        mc_t = iop.tile([P, UT * F], f32)

        nc.gpsimd.dma_start(out=Vst[:, 1:1 + F], in_=v0[:, :])
        nc.gpsimd.dma_start(out=M[:, 0:48], in_=m0[:, :])

        def halo_refresh(nblk):
            ps = psp.tile([P, 2], f32, tag="ps")
            c0 = Vst[:, nblk + 1:nblk + 2]
            c15 = Vst[:, nblk + 16:nblk + 17]
            nc.tensor.matmul(ps[:, 0:1], SH_L1, c15, start=True, stop=False)
            nc.tensor.matmul(ps[:, 0:1], SH_E00, c0, start=False, stop=True)
            nc.tensor.matmul(ps[:, 1:2], SH_R1, c0, start=True, stop=False)
            nc.tensor.matmul(ps[:, 1:2], SH_E127, c15, start=False, stop=True)
            nc.vector.tensor_copy(
                out=Vst[:, nblk:nblk + BLK:(BLK - 1)], in_=ps[:, 0:2])

        halo_refresh(0)

        def step(b):
            blk = b * BLK
            nblk = (b + 1) * BLK
            V = Vst[:, blk + 1:blk + 1 + F]
            VL = Vst[:, blk:blk + F]
            VR = Vst[:, blk + 2:blk + 2 + F]
            Vnew = Vst[:, nblk + 1:nblk + 1 + F]
            mh = (b % 2) * 48
            nh = 48 - mh
            m_g = M[:, mh + 0:mh + 16]
            h_g = M[:, mh + 32:mh + 48]
            Mcur = M[:, mh:mh + 48]
            Mnew = M[:, nh:nh + 48]

            ws = wsp.tile([P, 512], f32, tag="ws")
            Ap = ws[:, 0:48]        # [DTam, DTan, DTah]
            Bp = ws[:, 48:96]       # [DTbm, DTbn, DTbh]
            e15 = ws[:, 96:128]
            th = ws[:, 128:144]
            u12 = ws[:, 144:176]    # [DT*0.1*(V+40) | DT*0.01*(V+55)]
            D = ws[:, 176:208]
            R = ws[:, 208:240]
            S = ws[:, 240:288]
            G1 = ws[:, 288:336]
            Pg = ws[:, 336:384]
            m2 = ws[:, 384:400]
            mhp = ws[:, 400:416]
            ion2 = ws[:, 416:448]   # [m3h | n4]
            n2 = ws[:, 448:464]
            ve12 = ws[:, 464:496]   # -DT*[gna(V-E_NA) | gk(V-E_K)]
            # ISL: [-DT*I1 | -DT*I2 | -DT*I3 | V*(1-2*gax*DT)+DT*st+gax*DT*a1]
            ISL = wsp.tile([P, 64], f32, tag="isl")
            a1 = ws[:, 496:512]
            aux = wsp.tile([P, 48], f32, tag="aux")
            a2 = aux[:, 0:16]
            stD = aux[:, 16:32]
            A2m = aux[:, 32:48]
            mc = mc_t[:, b * F:(b + 1) * F]
            st = stim_t[:, b * F:(b + 1) * F]

            # scalar engine: 5 exps + tanh straight from V
            dsts = {"e1": e15[:, 0:16], "e5": e15[:, 16:32],
                    "DTah": Ap[:, 32:48], "DTbm": Bp[:, 0:16],
                    "DTbn": Bp[:, 16:32]}
            for key, scale, bias in EXPS:
                nc.scalar.activation(dsts[key], V, AF.Exp, bias=bias, scale=scale)
            nc.scalar.activation(th, V, AF.Tanh, bias=1.75, scale=0.05)

            # gate powers for this step's i_ion (from previous step's gates)
            nc.vector.tensor_mul(out=m2, in0=m_g, in1=m_g)
            nc.vector.tensor_mul(out=mhp, in0=m_g, in1=h_g)
            nc.vector.tensor_mul(out=ion2[:, 0:16], in0=m2, in1=mhp)
            nc.vector.tensor_mul(out=n2, in0=M[:, mh + 16:mh + 32],
                                 in1=M[:, mh + 16:mh + 32])
            nc.vector.tensor_mul(out=ion2[:, 16:32], in0=n2, in1=n2)
            # ion affines (feed the depth-3 chain; keep on DVE)
            nc.vector.tensor_scalar(out=ve12[:, 0:16], in0=V, scalar1=-DT * gna,
                                    scalar2=DT * gna * E_NA, op0=AluOp.mult,
                                    op1=AluOp.add)
            nc.vector.tensor_scalar(out=ve12[:, 16:32], in0=V, scalar1=-DT * gk,
                                    scalar2=DT * gk * E_K, op0=AluOp.mult,
                                    op1=AluOp.add)
            # affine prep (off critical chain)
            nc.vector.tensor_scalar(out=ISL[:, 32:48], in0=V, scalar1=-DT * gl,
                                    scalar2=DT * gl * E_L, op0=AluOp.mult,
                                    op1=AluOp.add)
            nc.gpsimd.tensor_scalar(out=stD, in0=st, scalar1=DT, scalar2=None,
                                    op0=AluOp.mult)
            nc.vector.tensor_scalar(out=u12[:, 0:16], in0=V, scalar1=0.1 * DT,
                                    scalar2=40.0 * 0.1 * DT, op0=AluOp.mult,
                                    op1=AluOp.add)
            nc.vector.tensor_scalar(out=u12[:, 16:32], in0=V, scalar1=0.01 * DT,
                                    scalar2=55.0 * 0.01 * DT, op0=AluOp.mult,
                                    op1=AluOp.add)
            nc.vector.tensor_scalar(out=Bp[:, 32:48], in0=th, scalar1=DT / 2,
                                    scalar2=DT / 2, op0=AluOp.mult,
                                    op1=AluOp.add)

            # V-critical chain, depth 3: (a1,a2) -> b1 -> reduce
            nc.vector.tensor_add(out=a1, in0=VL, in1=VR)
            nc.vector.scalar_tensor_tensor(
                out=a2, in0=V, scalar=1.0 - 2.0 * gax * DT, in1=stD,
                op0=AluOp.mult, op1=AluOp.add)
            nc.vector.scalar_tensor_tensor(
                out=ISL[:, 48:64], in0=a1, scalar=gax * DT, in1=a2,
                op0=AluOp.mult, op1=AluOp.add)
            nc.vector.tensor_mul(out=ISL[:, 0:32], in0=ion2, in1=ve12)
            nc.vector.tensor_reduce(
                out=Vnew, in_=ISL.rearrange("p (s f) -> p f s", s=4),
                axis=mybir.AxisListType.X, op=AluOp.add)
            # mem_cur output (off-chain): gax*a1 + (-2*gax*V + st)
            nc.vector.scalar_tensor_tensor(
                out=A2m, in0=V, scalar=-2.0 * gax, in1=st,
                op0=AluOp.mult, op1=AluOp.add)
            nc.vector.scalar_tensor_tensor(
                out=mc, in0=a1, scalar=gax, in1=A2m,
                op0=AluOp.mult, op1=AluOp.add)

            # rates / gates: M_new = (M + A') - (A'+B')*M
            nc.vector.tensor_scalar(out=D, in0=e15, scalar1=-1.0, scalar2=1.0,
                                    op0=AluOp.mult, op1=AluOp.add)
            nc.vector.reciprocal(R, D)
            nc.vector.tensor_mul(out=Ap[:, 0:32], in0=u12, in1=R)
            nc.vector.tensor_add(out=S, in0=Ap, in1=Bp)
            nc.vector.tensor_mul(out=Pg, in0=S, in1=Mcur)
            nc.vector.tensor_add(out=G1, in0=Mcur, in1=Ap)
            nc.vector.tensor_sub(out=Mnew, in0=G1, in1=Pg)
            halo_refresh(nblk)

        from contextlib import nullcontext

        rep_cm = tc.For_i(0, repeat) if repeat > 1 else None
        if rep_cm is not None:
            rep_cm.__enter__()
        for it_static in range(1 if loop else n_iter):
          with (tc.For_i(0, n_iter) if loop else nullcontext(it_static)) as it:
            H = UT // 2
            base = it * UT
            # stim in (two halves so compute can start after the first lands)
            nc.sync.dma_start(
                out=stim_t[:, 0:H * F],
                in_=stim[:, bass.ds(base, H), :])
            nc.sync.dma_start(
                out=stim_t[:, H * F:],
                in_=stim[:, bass.ds(base + H, H), :])
            for b in range(UT):
                step(b)
            # outputs
            nc.sync.dma_start(out=mc_out[:, bass.ds(base, H), :],
                              in_=mc_t[:, 0:H * F])
            nc.sync.dma_start(out=mc_out[:, bass.ds(base + H, H), :],
                              in_=mc_t[:, H * F:])
            vsrc = Vst[:, 0:UT * BLK].rearrange("p (t c) -> p t c", c=BLK)
            nc.sync.dma_start(out=v_out[:, bass.ds(base, UT), :],
                              in_=vsrc[:, :, 1:1 + F])
            # carry final state block to block 0
            nc.vector.tensor_copy(out=Vst[:, 0:BLK],
                                  in_=Vst[:, UT * BLK:(UT + 1) * BLK])
        if rep_cm is not None:
            rep_cm.__exit__(None, None, None)

        # final state out (UT even -> final gates in half 0)
        nc.sync.dma_start(out=vf_out[:, :], in_=Vst[:, 1:1 + F])
        nc.sync.dma_start(out=mf_out[:, :], in_=M[:, 0:48])

    _split_drain_waits(nc)
    return nc


def _numpy_fallback(inputs):
    f = np.float32
    V = inputs["V0"].astype(f).copy()
    m = inputs["m0"].astype(f).copy()
    h = inputs["h0"].astype(f).copy()
    n = inputs["n0"].astype(f).copy()
    g_na = inputs["g_na"].astype(f)
    g_k = inputs["g_k"].astype(f)
    g_l = inputs["g_l"].astype(f)
    g_ax = inputs["g_ax"].astype(f)
    stim = inputs["input_estim"].astype(f)
    T = stim.shape[0]
    mem_c = np.zeros_like(stim)
    mem_v = np.zeros_like(stim)
    one = f(1.0)
    dt = f(DT)
    for t in range(T):
        am = f(0.1) * (V + f(40.0)) / (one - np.exp(-(V + f(40.0)) / f(10.0)))
        bm = f(4.0) * np.exp(-(V + f(65.0)) / f(18.0))
        ah = f(0.07) * np.exp(-(V + f(65.0)) / f(20.0))
        bh = one / (one + np.exp(-(V + f(35.0)) / f(10.0)))
        an = f(0.01) * (V + f(55.0)) / (one - np.exp(-(V + f(55.0)) / f(10.0)))
        bn = f(0.125) * np.exp(-(V + f(65.0)) / f(80.0))
        i_ion = (g_na * m**3 * h * (V - f(E_NA)) + g_k * n**4 * (V - f(E_K))
                 + g_l * (V - f(E_L)))
        Vp = np.pad(V, (1, 1), mode="edge")
        i_ax = g_ax * (Vp[:-2] + Vp[2:] - f(2.0) * V)
        dV = i_ax + stim[t] - i_ion
        mem_c[t] = dV + i_ion
        mem_v[t] = V
        V = V + dt * dV
        m = m + dt * (am * (one - m) - bm * m)
        h = h + dt * (ah * (one - h) - bh * h)
        n = n + dt * (an * (one - n) - bn * n)
    return mem_c, mem_v, V, m, h, n


def _shift_mats():
    L1 = np.eye(P, k=1, dtype=np.float32)      # lhsT for left halo shift
    E00 = np.zeros((P, P), np.float32); E00[0, 0] = 1.0
    R1 = np.eye(P, k=-1, dtype=np.float32)
    E127 = np.zeros((P, P), np.float32); E127[P - 1, P - 1] = 1.0
    return np.concatenate([L1, E00, R1, E127], axis=1)  # (P, 4P)


def _make_in_map(inputs, T_total):
    stim = np.ascontiguousarray(
        np.asarray(inputs["input_estim"], np.float32)
        .reshape(T_total, P, F).transpose(1, 0, 2))
    m0 = np.concatenate([
        np.asarray(inputs["m0"], np.float32).reshape(P, F),
        np.asarray(inputs["n0"], np.float32).reshape(P, F),
        np.asarray(inputs["h0"], np.float32).reshape(P, F)], axis=1)
    return {
        "stim": stim,
        "v0": np.asarray(inputs["V0"], np.float32).reshape(P, F),
        "m0": np.ascontiguousarray(m0),
        "shifts": _shift_mats(),
    }


def _run_on_hw(inputs, T_total, UT, trace=False):
    from concourse.bass_utils import run_bass_kernel_spmd

    g = {}
    for key, name in (("g_na", "gna"), ("g_k", "gk"), ("g_l", "gl"),
                      ("g_ax", "gax")):
        arr = np.asarray(inputs[key], np.float32)
        if not np.all(arr == arr.flat[0]):
            return None  # non-uniform: caller falls back
        g[name] = float(arr.flat[0])

    ck = (tuple(sorted(g.items())), T_total, UT)
    if ck not in _PROGRAM_CACHE:
        _PROGRAM_CACHE[ck] = _build_program(g, T_total, UT)
    nc = _PROGRAM_CACHE[ck]
    in_map = _make_in_map(inputs, T_total)
    res = run_bass_kernel_spmd(nc, [in_map] * 8, core_ids=list(range(8)),
                               trace=trace)
    r = res.results[0]
    mem_c = r["mc"].transpose(1, 0, 2).reshape(T_total, N_SEG)
    mem_v = r["vout"].transpose(1, 0, 2).reshape(T_total, N_SEG)
    Vf = r["vf"].reshape(N_SEG)
    mf = r["mf"][:, 0:16].reshape(N_SEG)
    nf = r["mf"][:, 16:32].reshape(N_SEG)
    hf = r["mf"][:, 32:48].reshape(N_SEG)
    out = (mem_c, mem_v, Vf, mf, hf, nf)
    if trace:
        return out, res
    return out


def kernel(**inputs):
    T_total = int(np.asarray(inputs["input_estim"]).shape[0])
    UT = 128
    if T_total % UT != 0:
        for cand in (64, 32, 16, 8, 4, 2):
            if T_total % cand == 0:
                UT = cand
                break
        else:
            return _numpy_fallback(inputs)
    out = _run_on_hw(inputs, T_total, UT)
    if out is None:
        return _numpy_fallback(inputs)
    return out
